# revision 9
# baseline (speedup 1.0000x reference)
import os

os.environ.setdefault("NEURON_CC_FLAGS", "--auto-cast=none")

import numpy as np
import jax
import jax.numpy as jnp

# Problem constants (nn_GatLayer_59167469470141): B=8192 dst nodes, N=64
# neighbors, F=32 features, 8 cores, shard along B (1024 dst nodes/core).
SIGMA = 1.0
THRESH = 0.35
MAX_ITERS = 48
# The greedy loop's global stop fires after 4 iterations on this data (the
# global max gain is non-increasing, so once it dips under THRESH it stays
# under). We run a fixed T_RUN iterations on device, emit per-iteration
# prefix results + per-iteration max gains, and pick the exact stop
# iteration K on the host (comparisons only, no arithmetic).
T_RUN = 5
N_CORES = 8


def _core(mail, src_norm, dst_norm, attn_w):
    # mail [b,64,32], src_norm [b,64], dst_norm [b], attn_w [32,1]
    feat = mail * src_norm[..., None]
    sq = jnp.sum(feat * feat, axis=-1)                       # [b,64]
    # PE matmul; its fp32 accumulation noise (~1e-6 rel) is far below the
    # host-side ambiguity net (5e-3), so borderline argmax rows are safe.
    dot = jnp.einsum("bnf,bmf->bnm", feat, feat)
    d2 = sq[:, :, None] + sq[:, None, :] - 2.0 * dot
    dists = jnp.sqrt(jnp.maximum(d2, 0.0))
    mean_d = dists.mean(axis=(-2, -1))[:, None, None]
    sims = jnp.exp(-dists / (SIGMA * mean_d))                # [b,64,64]

    logits = jnp.einsum("bnf,fo->bn", feat, attn_w)
    attention = jax.nn.softmax(logits, axis=1)               # [b,64]

    b = feat.shape[0]
    n = feat.shape[1]
    iota = jnp.arange(n)[None, :]                            # [1,64]

    cache = jnp.zeros((b, n), feat.dtype)
    acc = jnp.zeros((b, feat.shape[2]), feat.dtype)
    snaps = []
    wgs = []
    for _ in range(T_RUN):
        gain = jnp.sum(
            jnp.maximum(sims, cache[:, None, :]) - cache[:, None, :], axis=-1
        ) * attention                                        # [b,64]
        sel = jnp.argmax(gain, axis=1)                       # [b]
        onehot = (iota == sel[:, None]).astype(feat.dtype)   # [b,64]
        g1 = gain.max(axis=1)
        g2 = (gain - onehot * jnp.float32(1e30)).max(axis=1)
        wgs.append(jnp.stack([g1, g2], axis=-1))             # [b,2]
        row = jnp.einsum("bn,bnj->bj", onehot, sims)         # sims[b,sel,:]
        frow = jnp.einsum("bn,bnf->bf", onehot, feat)        # feat[b,sel,:]
        acc = acc + frow
        cache = jnp.maximum(cache, row)
        snaps.append(acc * dst_norm[:, None])
    return jnp.stack(snaps, axis=1), jnp.stack(wgs, axis=1)  # [b,T,32],[b,T,2]


_pcore = jax.pmap(_core, in_axes=(0, 0, 0, None), static_broadcasted_argnums=())


def _reference_fallback(mail, attn_w, src_norm, dst_norm):
    # Exact numpy replica of the reference greedy loop; only used if the
    # global stop has not fired within T_RUN iterations (never on the
    # shipped dataset).
    feat = mail * src_norm[..., None]
    B, N, F = feat.shape
    sq = np.sum(feat * feat, axis=-1)
    d2 = sq[:, :, None] + sq[:, None, :] - 2.0 * np.einsum(
        "bnf,bmf->bnm", feat, feat
    )
    dists = np.sqrt(np.maximum(d2, 0.0))
    mean_d = dists.mean(axis=(-2, -1))[:, None, None]
    sims = np.exp(-dists / (SIGMA * mean_d))
    logits = np.einsum("bnf,fo->bn", feat, attn_w)
    z = np.exp(logits - logits.max(1, keepdims=True))
    att = z / z.sum(1, keepdims=True)
    bidx = np.arange(B)
    cache = np.zeros((B, N), np.float32)
    acc = np.zeros((B, F), np.float32)
    active = True
    for _ in range(MAX_ITERS):
        gain = (
            np.sum(np.maximum(sims, cache[:, None, :]) - cache[:, None, :], -1)
            * att
        )
        mv = gain.max()
        sel = np.argmax(gain, axis=1)
        if active:
            acc += feat[bidx, sel]
            cache = np.maximum(sims[bidx, sel], cache)
        active = active and (mv >= THRESH)
    return (acc * dst_norm[:, None]).astype(np.float32)


def _exact_rows(mail, attn_w, src_norm, dst_norm, K):
    # Reference-exact fp32 greedy for a small subset of rows, running
    # exactly K iterations (the globally-gated schedule is shared).
    feat = mail * src_norm[..., None]
    B, N, F = feat.shape
    sq = np.sum(feat * feat, axis=-1)
    d2 = sq[:, :, None] + sq[:, None, :] - 2.0 * np.einsum(
        "bnf,bmf->bnm", feat, feat
    )
    dists = np.sqrt(np.maximum(d2, 0.0))
    mean_d = dists.mean(axis=(-2, -1))[:, None, None]
    sims = np.exp(-dists / (SIGMA * mean_d))
    logits = np.einsum("bnf,fo->bn", feat, attn_w)
    z = np.exp(logits - logits.max(1, keepdims=True))
    att = z / z.sum(1, keepdims=True)
    bidx = np.arange(B)
    cache = np.zeros((B, N), np.float32)
    acc = np.zeros((B, F), np.float32)
    for _ in range(K):
        gain = (
            np.sum(np.maximum(sims, cache[:, None, :]) - cache[:, None, :], -1)
            * att
        )
        sel = np.argmax(gain, axis=1)
        acc += feat[bidx, sel]
        cache = np.maximum(sims[bidx, sel], cache)
    return (acc * dst_norm[:, None]).astype(np.float32)


def kernel(mail, attn_w, src_norm, dst_norm):
    mail = np.asarray(mail, np.float32)
    attn_w = np.asarray(attn_w, np.float32)
    src_norm = np.asarray(src_norm, np.float32)
    dst_norm = np.asarray(dst_norm, np.float32)
    B = mail.shape[0]
    bs = B // N_CORES

    m = mail.reshape(N_CORES, bs, *mail.shape[1:])
    s = src_norm.reshape(N_CORES, bs, src_norm.shape[1])
    d = dst_norm.reshape(N_CORES, bs)

    snaps, wgs = _pcore(m, s, d, attn_w)
    snaps = np.asarray(snaps)                 # [8, bs, T_RUN, 32]
    wgs = np.asarray(wgs)                     # [8, bs, T_RUN, 2]

    # Host: exact global stop logic (comparisons only). active_0=True;
    # iteration t contributes iff active_t; active_{t+1} = active_t and
    # (max gain_t >= THRESH).
    g = wgs[..., 0].max(axis=(0, 1))          # [T_RUN] global max per iter
    K = 0
    active = True
    for t in range(T_RUN):
        if active:
            K = t + 1
        active = active and (g[t] >= THRESH)
    if active and T_RUN < MAX_ITERS:
        # Stop never fired within T_RUN — fall back to the exact loop.
        return _reference_fallback(mail, attn_w, src_norm, dst_norm)

    out = snaps[:, :, K - 1, :].reshape(B, -1)
    out = np.ascontiguousarray(out, dtype=np.float32)

    # Rows whose argmax was decided by a gap smaller than device fp noise
    # can differ from the fp32 reference trajectory; recompute those few
    # rows with the reference-exact path.
    g1 = wgs[..., 0].reshape(B, T_RUN)[:, :K]
    g2 = wgs[..., 1].reshape(B, T_RUN)[:, :K]
    amb = ((g1 - g2) < 5e-3 * np.abs(g1) + 1e-7).any(axis=1)
    idx = np.nonzero(amb)[0]
    if idx.size:
        out[idx] = _exact_rows(
            mail[idx], attn_w, src_norm[idx], dst_norm[idx], K
        )
    return out
